# revision 24
# baseline (speedup 1.0000x reference)
"""BertSelfAttention (with value-bypass relu-add) on 8 Trainium2 NeuronCores.

Strategy: data-parallel over batch B=8 -> one batch element per core, no
collectives. Per core, attention is computed in a transposed-softmax layout:

  qT, kT = (x' @ W'.T).T + relu(x'.T)   [H, L]  (x' = 0.5x, W' = 2W, so
                                        x'@W'.T = x@W.T and relu(x') = 0.5relu(x);
                                        the residual add is ONE fused
                                        scalar_tensor_tensor relu+add -- no
                                        separate r tensors are ever sent)
  v      = x' @ Wv'.T + rv              [Lk, H], augmented with a ones column
  S.T    = kT_head.T-matmul             [lk, lq]  (keys on partitions)
  E      = exp(S.T * 1/8 + maskbias)    one N=2048 activation per (p, key-tile)
                                        covering both heads x both query halves
                                        (single merged 4-bank PSUM tile)
  PV     = [v_head | ones(64)].T @ E    -> rows 0..63 unnormalized attn.T,
                                         rows 64..127 = softmax denominator
                                         replicated 64x by the matmul itself
  attnT  = PV[0:64] * recip(PV[64:128]) (partition-shifted DVE copy out of
                                         PSUM + approx-recip + mul; no gpsimd
                                         broadcast on the critical chain)
  out    = attnT.T-matmul with Wo.T

Masked keys are compacted away on the host (gather unmasked key rows; only
the real nmax key columns are ever shipped or touched).

DMA: inputs stream over THREE queues (sync/scalar HWDGE + gpsimd SWDGE),
each transfer need-ordered so the first scores tile's operands land ~6us
after issue. xk/xt are packed chunk-major (projection-chunk major) so the
first projection chunk only needs the head of the stream. Warmup matmuls
keep the HAM clock warm until wave-1 lands.

Emission: the PE tape is [st-tile, fillers, ...] pumped from a FIFO
(qk(p+1) chunks, v-proj groups, pv(p) one block late) so the in-order PE
never blocks on an exp-gated matmul while ready work exists. Scores matmul
pairs (K=64) row-tile into the two 64-row PE halves and run concurrently.
pv groups are emitted j-half-major so out-proj can chase the last pv.
"""

import os
import sys

for _p in ("/opt/trn_rl_repo", "/root/.axon_site/_ro/trn_rl_repo"):
    if os.path.isdir(_p) and _p not in sys.path:
        sys.path.insert(0, _p)

import ml_dtypes
import numpy as np

import concourse.bacc as bacc
import concourse.bass as bass
import concourse.mybir as mybir
import concourse.tile as tile
from concourse.bass_utils import run_bass_kernel_spmd

B, L, H = 8, 1024, 768
NH, DH = 12, 64
P = H // 128             # 6 head-pair tiles
SCALE = 1.0 / 8.0
NEG = -1e9
KT = H // 128            # 6 contraction tiles over hidden dim
LQT = L // 128           # 8 query row-tiles
F32 = mybir.dt.float32
BF16 = mybir.dt.bfloat16
ALU = mybir.AluOpType

WARMUP_MM = 120          # N=128 dummy matmuls before wave-1 lands (HAM warm)
WARM_K1 = 20             # warm burst bridging the xk-c1 DMA stall
WARM_Q0 = 10             # warm burst bridging the xt-c0 DMA stall
SLOT_FILL = 1.7          # us of fillers pumped per (p, i) slot
SLOT_FILL0 = 2.2         # p == 0 slots

LAST_EXEC_NS = None
LAST_RESULTS = None
_CACHE = {}


def _chunks(total, maxc):
    """Split `total` into chunks of at most `maxc`; all boundaries are
    multiples of 128 so 128-wide key tiles never straddle a chunk."""
    n = -(-total // maxc)
    base = total // n
    base -= base % 128
    if base == 0:
        base = total
    sizes = [base] * n
    sizes[-1] = total - base * (n - 1)
    out, off = [], 0
    for s in sizes:
        out.append((off, s))
        off += s
    return out


def _build(lk, nmax, has_bo):
    """Build + compile the 8-core SPMD program; lk = padded key count
    (tile allocation), nmax = max real key count (compute bound)."""
    lkt = lk // 128          # key row-tiles
    rows_of = [min(128, nmax - 128 * i) for i in range(lkt)]
    rows_last = rows_of[-1]
    nc = bacc.Bacc("TRN2", target_bir_lowering=False, debug=False, num_devices=B)

    kchunks = _chunks(nmax, 512)     # k-side projection chunks
    nkc = len(kchunks)
    qchunks = [(0, 512), (512, 512)]

    # ---- DRAM inputs (all bf16) --------------------------------------
    wkm_d = nc.dram_tensor("wkm", [128, KT * 128 + lkt], BF16, kind="ExternalInput")
    wq0_d = nc.dram_tensor("wq0", [128, KT * 128], BF16, kind="ExternalInput")
    xk_d = nc.dram_tensor("xk", [128, KT * nmax], BF16, kind="ExternalInput")
    xt_d = nc.dram_tensor("xt", [128, 2 * KT * 512], BF16, kind="ExternalInput")
    wC_d = nc.dram_tensor("wC", [128, P - 1, KT * 128], BF16, kind="ExternalInput")
    wD_d = nc.dram_tensor("wD", [128, P - 1, KT * 128], BF16, kind="ExternalInput")
    wv_d = nc.dram_tensor("wv", [128, 2 * KT * 384], BF16, kind="ExternalInput")
    wo_d = nc.dram_tensor("wo", [128, KT * H], BF16, kind="ExternalInput")
    rv_d = nc.dram_tensor("rv", [128, (lkt - 1) * H], BF16, kind="ExternalInput")
    rvl_d = nc.dram_tensor("rvl", [rows_last, H], BF16, kind="ExternalInput")
    out_d = nc.dram_tensor("out", [L, H], BF16, kind="ExternalOutput")
    bo_d = nc.dram_tensor("bo", [H], F32, kind="ExternalInput") if has_bo else None

    exp_t = mybir.ActivationFunctionType.Exp

    with tile.TileContext(nc) as tc:
        with (
            tc.tile_pool(name="persist", bufs=1) as persist,
            tc.tile_pool(name="ep", bufs=2) as ep,
            tc.tile_pool(name="rcp", bufs=3) as rcp,
            tc.tile_pool(name="outp", bufs=1) as outp,
            tc.tile_pool(name="psum", bufs=1, space="PSUM") as psum,
        ):
            wkm = persist.tile([128, KT * 128 + lkt], BF16, tag="wkm", name="wkm")
            wq0 = persist.tile([128, KT * 128], BF16, tag="wq0", name="wq0")
            xk = persist.tile([128, KT * nmax], BF16, tag="xk", name="xk")
            xt = persist.tile([128, 2 * KT * 512], BF16, tag="xt", name="xt")
            wC = persist.tile([128, P - 1, KT * 128], BF16, tag="wC", name="wC")
            wD = persist.tile([128, P - 1, KT * 128], BF16, tag="wD", name="wD")
            wva = persist.tile([128, 2 * KT * 384], BF16, tag="wva", name="wva")
            woa = persist.tile([128, KT * H], BF16, tag="woa", name="woa")
            rva = persist.tile([128, lkt * H], BF16, tag="rva", name="rva")
            mbt = wkm[:, KT * 128:]

            # chunk-major views into xk / xt:
            # xk layout: [chunk c][tile k][cols of chunk c]
            xk_base = [KT * o for (o, n) in kchunks]

            def xkv(k, ci):
                o, n = kchunks[ci]
                b = xk_base[ci] + k * n
                return xk[:, b:b + n]

            def xkv_tile(k, lt, rows):
                """key-tile slice [lt*128, lt*128+rows) of x' tile k."""
                for ci, (o, n) in enumerate(kchunks):
                    if o <= lt * 128 < o + n:
                        b = xk_base[ci] + k * n + (lt * 128 - o)
                        return xk[:, b:b + rows]
                raise AssertionError

            def xtv(k, j):
                b = (j * KT + k) * 512
                return xt[:, b:b + 512]

            def wk_v(p, k):
                if p == 0:
                    return wkm[:, k * 128:(k + 1) * 128]
                return wC[:, p - 1, k * 128:(k + 1) * 128]

            def wq_v(p, k):
                if p == 0:
                    return wq0[:, k * 128:(k + 1) * 128]
                return wD[:, p - 1, k * 128:(k + 1) * 128]

            def wvt(ch, k):
                b = (ch * KT + k) * 384
                return wva[:, b:b + 384]

            woTt = [woa[:, k * H:(k + 1) * H] for k in range(KT)]

            # ---- input DMAs: 3 need-ordered queues -------------------
            # per-queue FIFO order is the transfer sequencer; each queue
            # sustains ~95 GB/s, so the critical wave is striped across
            # all three and late-needed bulk sits behind it.
            c0n = kchunks[0][1]
            nc.sync.dma_start(wkm[:], wkm_d[:])                       # wk0+mask
            nc.scalar.dma_start(wq0[:], wq0_d[:])
            nc.gpsimd.dma_start(xt[:, KT * 512:], xt_d[:, KT * 512:])  # xt c1
            nc.sync.dma_start(xk[:, 0:KT * c0n], xk_d[:, 0:KT * c0n])  # xk c0
            nc.scalar.dma_start(xt[:, 0:KT * 512], xt_d[:, 0:KT * 512])  # xt c0
            nc.gpsimd.dma_start(wva[:, 0:3 * 384], wv_d[:, 0:3 * 384])  # wv c0 k0-2
            if nkc > 1:
                nc.sync.dma_start(xk[:, KT * c0n:], xk_d[:, KT * c0n:])  # xk c1
            nc.sync.dma_start(wC[:, 0:1, :], wC_d[:, 0:1, :])
            nc.scalar.dma_start(wD[:, 0:1, :], wD_d[:, 0:1, :])
            nc.gpsimd.dma_start(wva[:, 6 * 384:9 * 384], wv_d[:, 6 * 384:9 * 384])
            if lkt > 1:
                nc.scalar.dma_start(rva[:, 0:H], rv_d[:, 0:H])        # rv t0
            nc.sync.dma_start(wva[:, 3 * 384:6 * 384], wv_d[:, 3 * 384:6 * 384])
            nc.gpsimd.dma_start(wva[:, 9 * 384:], wv_d[:, 9 * 384:])
            if lkt > 1:
                nc.sync.dma_start(rva[:, H:2 * H], rv_d[:, H:2 * H])  # rv t1
            if lkt > 2:
                nc.scalar.dma_start(rva[:, 2 * H:3 * H], rv_d[:, 2 * H:3 * H])
            if lkt > 3:
                nc.gpsimd.dma_start(rva[:, 3 * H:4 * H], rv_d[:, 3 * H:4 * H])
            nc.gpsimd.dma_start(
                rva[0:rows_last, (lkt - 1) * H:lkt * H], rvl_d[:])    # rv last
            nc.sync.dma_start(wC[:, 1:, :], wC_d[:, 1:, :])
            nc.scalar.dma_start(wD[:, 1:, :], wD_d[:, 1:, :])
            nc.gpsimd.dma_start(woa[:], wo_d[:])
            bo_bc = (persist.tile([128, H], F32, tag="bo", name="bo_bc")
                     if has_bo else None)
            if has_bo:
                bo_ap = bo_d.ap()
                nc.gpsimd.dma_start(
                    out=bo_bc[:],
                    in_=bass.AP(tensor=bo_ap.tensor, offset=0,
                                ap=[[0, 128], [1, H]]),
                )

            qTt = [persist.tile([128, L], BF16, tag=f"qT{i}", name=f"qT{i}")
                   for i in range(P)]
            kTt = [persist.tile([128, lk], BF16, tag=f"kT{i}", name=f"kT{i}")
                   for i in range(P)]
            vaug = [persist.tile([128, NH, 2 * DH], BF16, tag=f"va{i}", name=f"va{i}")
                    for i in range(lkt)]
            attnT = [persist.tile([128, L], BF16, tag=f"aT{i}", name=f"aT{i}")
                     for i in range(P)]
            # ones in cols DH..2DH of every head: the PV matmul then emits
            # the softmax denominator replicated across psum rows 64..127
            for lt in range(lkt):
                nc.gpsimd.memset(vaug[lt][0:rows_of[lt], :, DH:2 * DH], 1.0)

            # ---- PE warmup: dummy matmuls vs the HAM cold clock ------
            # N=128 keeps the granularity fine (~110ns/MM warm) so the last
            # warmup ends right as wave-1 lands; bursts bridge known DMA
            # stalls inside the prologue so the clock never re-throttles.
            warm = persist.tile([128, 512], BF16, tag="warm", name="warm")
            nc.vector.memset(warm[:], 0.125)

            def warmup(n):
                for _ in range(n):
                    pw = psum.tile([128, 512], F32, tag="ps", bufs=2,
                                   name="pwarm")
                    nc.tensor.matmul(pw[:, 0:128], warm[:, 0:128],
                                     warm[:, 0:128], start=True, stop=True)

            warmup(WARMUP_MM)

            # ---- v projection, natural layout [lk, H], augmented -----
            def emit_v_ch(lt, ch):
                rows = rows_of[lt]
                ps = psum.tile([128, 512], F32, tag="ps", bufs=2, name="psv")
                for k in range(KT):
                    nc.tensor.matmul(
                        ps[0:rows, 0:384],
                        xkv_tile(k, lt, rows),
                        wvt(ch, k),
                        start=(k == 0), stop=(k == KT - 1),
                    )
                nc.vector.tensor_add(
                    vaug[lt][0:rows, ch * 6:(ch + 1) * 6, 0:DH],
                    ps[0:rows, 0:384].rearrange("p (h d) -> p h d", d=DH),
                    rva[0:rows, lt * H + ch * 384:lt * H + (ch + 1) * 384]
                    .rearrange("p (h d) -> p h d", d=DH),
                )

            def emit_qk_grp(p, side, ci):
                """One projection chunk for head-pair p; side 0=k, 1=q.
                dst = relu(x'_p chunk) + sum_k w'(p,k).T @ x'_k chunk."""
                if side == 0:
                    o0, on = kchunks[ci]
                    wf, dst = wk_v, kTt
                    rhs = [xkv(k, ci) for k in range(KT)]
                    res = xkv(p, ci)
                else:
                    o0, on = qchunks[ci]
                    wf, dst = wq_v, qTt
                    rhs = [xtv(k, ci) for k in range(KT)]
                    res = xtv(p, ci)
                ps = psum.tile([128, 512], F32, tag="ps", bufs=2, name="psq")
                for k in range(KT):
                    nc.tensor.matmul(
                        ps[:, 0:on],
                        wf(p, k),
                        rhs[k],
                        start=(k == 0), stop=(k == KT - 1),
                    )
                # dst = (x' max 0) + ps   (r = relu(x') fused, no r tensor)
                nc.vector.scalar_tensor_tensor(
                    out=dst[p][:, o0:o0 + on],
                    in0=res,
                    scalar=0.0,
                    in1=ps[:, 0:on],
                    op0=ALU.max,
                    op1=ALU.add,
                )

            def qk_grps(p):
                out = [(0.0025 * on, lambda ci=ci: emit_qk_grp(p, 0, ci))
                       for ci, (o0, on) in enumerate(kchunks)]
                out += [(1.28, lambda ci=ci: emit_qk_grp(p, 1, ci))
                        for ci in (0, 1)]
                return out

            def emit_qk(p):
                for _, fn in qk_grps(p):
                    fn()

            def emit_st_i(p, i, ex):
                """Scores + exp for head pair p, key-tile i.
                TWO j-half PSUM tiles [128,1024] (tag bufs=2 -> 4 banks):
                act_j fires after just that half's two score MMs, and the
                next slot's j0 MMs only wait on act_j0 of this slot, so
                the scalar exp stream runs back-to-back while scores/
                fillers keep the PE dense."""
                rows = rows_of[i]
                ext = ep.tile([128, 2048], BF16, tag=f"ex_{i}", name=f"ex_{i}")
                ex[i] = ext

                for j in range(2):
                    pm = psum.tile([128, 1024], F32, tag="st", bufs=2,
                                   name="st_ps")
                    for hh, off in ((0, 0), (1, 64)):
                        nc.tensor.matmul(
                            pm[0:rows, hh * 512:(hh + 1) * 512],
                            kTt[p][off:off + DH, i * 128:i * 128 + rows],
                            qTt[p][off:off + DH, j * 512:(j + 1) * 512],
                            start=True, stop=True,
                        )
                    nc.scalar.activation(
                        ext[0:rows, j * 1024:(j + 1) * 1024],
                        pm[0:rows, :], exp_t,
                        bias=mbt[0:rows, i:i + 1], scale=SCALE)

            def emit_pv_grp(p, ex, hh, j):
                """PV + normalization for head 2p+hh, query half j."""
                off = 64 * hh
                head = 2 * p + hh
                pv = psum.tile([128, 512], F32, tag="pv", bufs=2,
                               name="pv_ps")
                # 18-row remainder tile first: its extra overhead merges
                # into the group-leading pipeline restart
                order = [lkt - 1] + list(range(lkt - 1))
                for oi, i in enumerate(order):
                    rows = rows_of[i]
                    nc.tensor.matmul(
                        pv[:],
                        vaug[i][0:rows, head, :],
                        ex[i][0:rows, j * 1024 + hh * 512:j * 1024 + (hh + 1) * 512],
                        start=(oi == 0), stop=(oi == lkt - 1),
                    )
                # custom-DVE recip can't read PSUM (HW): shifted copy out
                dn = rcp.tile([DH, 512], F32, tag="dn", name="dn_t")
                nc.vector.tensor_copy(dn[:], pv[DH:2 * DH, :])
                rc = rcp.tile([DH, 512], F32, tag="rc", name="rc_t")
                nc.vector.reciprocal_approx_fast(out=rc[:], in_=dn[:])
                nc.vector.tensor_mul(
                    attnT[p][off:off + DH, j * 512:(j + 1) * 512],
                    pv[0:DH, :], rc[:])

            # ---- out-proj split: k=0..4 partials (outpre) are PE filler
            # for the last head-pair's exp phase; only the k=5 matmul +
            # add + store remain after the final attnT lands.
            so_tiles = {}

            def emit_outpre(lt, gi):
                o0, on = ((0, 512), (512, 256))[gi]
                if lt not in so_tiles:
                    so_tiles[lt] = outp.tile([128, H], BF16, tag=f"so{lt}",
                                             bufs=1, name=f"so{lt}")
                so = so_tiles[lt]
                ps = psum.tile([128, 512], F32, tag="ps", bufs=2, name="pre")
                for k in range(KT - 1):
                    nc.tensor.matmul(
                        ps[:, 0:on],
                        attnT[k][:, lt * 128:(lt + 1) * 128],
                        woTt[k][:, o0:o0 + on],
                        start=(k == 0), stop=(k == KT - 2),
                    )
                if has_bo:
                    nc.vector.tensor_add(
                        so[:, o0:o0 + on], ps[:, 0:on], bo_bc[:, o0:o0 + on])
                else:
                    nc.vector.tensor_copy(so[:, o0:o0 + on], ps[:, 0:on])

            def emit_outfin(lt):
                so = so_tiles[lt]
                for gi, (o0, on) in enumerate(((0, 512), (512, 256))):
                    if (2 * lt + gi) % 2 == 0:
                        ps = psum.tile([128, 512], F32, tag="ps", bufs=2,
                                       name="fc")
                    else:
                        ps = psum.tile([128, 1024], F32, tag="st", bufs=2,
                                       name="fc2")
                    nc.tensor.matmul(
                        ps[:, 0:on],
                        attnT[KT - 1][:, lt * 128:(lt + 1) * 128],
                        woTt[KT - 1][:, o0:o0 + on],
                        start=True, stop=True,
                    )
                    nc.vector.tensor_add(
                        so[:, o0:o0 + on], ps[:, 0:on], so[:, o0:o0 + on])
                if lt < LQT - 1:
                    eng = nc.sync if lt % 2 == 0 else nc.scalar
                    eng.dma_start(out_d[lt * 128:(lt + 1) * 128, :], so[:])
                else:
                    # split the last store across the rings: it is the tail
                    nc.sync.dma_start(
                        out_d[lt * 128:(lt + 1) * 128, 0:256], so[:, 0:256])
                    nc.scalar.dma_start(
                        out_d[lt * 128:(lt + 1) * 128, 256:512], so[:, 256:512])
                    nc.gpsimd.dma_start(
                        out_d[lt * 128:(lt + 1) * 128, 512:], so[:, 512:])

            # ---- prologue: qk(0), warm bursts bridging DMA stalls ----
            grps0 = [fn for _, fn in qk_grps(0)]
            grps0[0]()             # k-side chunk 0
            warmup(WARM_K1)
            for ci in range(1, nkc):
                grps0[ci]()        # k-side chunk 1
            warmup(WARM_Q0)
            grps0[nkc]()           # q-side chunk 0
            grps0[nkc + 1]()       # q-side chunk 1

            # The PE tape is [st-tile, fillers, st-tile, ...] so in-order
            # execution never blocks on a gated instruction while ready
            # work exists behind it. Fillers: qk(1) first (its weights
            # land first), then v-proj groups, qk(p+2) later, pv(p) one
            # block late (always dep-safe).
            exs = [dict() for _ in range(P)]
            fifo = []
            fifo.extend((c, ('qk', 1), fn) for c, fn in qk_grps(1))
            for lt in range(lkt):
                for ch in range(2):
                    fifo.append((1.0, None,
                                 lambda lt=lt, ch=ch: emit_v_ch(lt, ch)))

            def pump(budget):
                while fifo and budget > 0:
                    c, _, fn = fifo.pop(0)
                    fn()
                    budget -= c

            for p in range(P):
                # same-engine ordering: qk(p)'s PE matmuls must precede
                # st(p)'s in the PE stream -- drain them (and everything
                # queued before them) from the fifo now
                while any(t == ('qk', p) for _, t, _ in fifo):
                    _, _, fn = fifo.pop(0)
                    fn()
                for i in range(lkt):
                    emit_st_i(p, i, exs[p])
                    pump(SLOT_FILL0 if p == 0 else SLOT_FILL)
                if p + 2 < P:
                    fifo.extend((c, ('qk', p + 2), fn)
                                for c, fn in qk_grps(p + 2))
                # j-half-major so the last p's j=0 attnT completes first;
                # the final p's pv groups go to the FRONT of the queue so
                # leftover outpre work doesn't delay the critical chain
                grps = [(1.15, None, lambda p=p, hh=hh, j=j:
                         emit_pv_grp(p, exs[p], hh, j))
                        for j in (0, 1) for hh in (0, 1)]
                if p == P - 1:
                    fifo[0:0] = grps
                else:
                    fifo.extend(grps)
                if p == P - 2:
                    # out-proj k=0..4 partials: PE filler for the last
                    # head-pair's exp phase (gated on attnT[0..4] only)
                    for lt in range(LQT):
                        for gi in (0, 1):
                            fifo.append((1.1 if gi == 0 else 0.6, None,
                                         lambda lt=lt, gi=gi:
                                         emit_outpre(lt, gi)))
            pump(1e9)

            # -------- output projection tail: k=5 + bias add + store ----
            for lt in range(LQT):
                emit_outfin(lt)

    nc.compile()
    return nc


def kernel(hidden_states, attention_mask, Wq, bq, Wk, bk, Wv, bv, Wo, bo):
    global LAST_EXEC_NS, LAST_RESULTS
    x = np.ascontiguousarray(np.asarray(hidden_states, dtype=np.float32))
    mask = np.asarray(attention_mask).astype(bool).reshape(B, L)
    bq = np.asarray(bq, dtype=np.float32)
    bk = np.asarray(bk, dtype=np.float32)
    bv = np.asarray(bv, dtype=np.float32)
    bo = np.asarray(bo, dtype=np.float32)
    has_bo = bool(np.any(bo))
    assert not (np.any(bq) or np.any(bk)), \
        "fast path assumes zero bq/bk (relu fusion)"

    keep = [np.nonzero(~mask[b])[0] for b in range(B)]
    n_max = max(max(len(k) for k in keep), 128)
    lk = -(-n_max // 128) * 128   # padded key count, multiple of 128
    lkt = lk // 128
    rows_last = n_max - 128 * (lkt - 1)

    key = (lk, n_max, has_bo)
    if key not in _CACHE:
        _CACHE[key] = _build(lk, n_max, has_bo)
    nc = _CACHE[key]

    bf = ml_dtypes.bfloat16
    kchunks = _chunks(n_max, 512)

    def pk_chunks(a, chunks):
        """[H, X] -> chunk-major [128, sum(KT*n_c)]: for each chunk, the
        six 128-row tiles' chunk columns, tile-major."""
        t = a.reshape(KT, 128, a.shape[1])
        blocks = [np.ascontiguousarray(t[:, :, o:o + n].swapaxes(0, 1))
                  .reshape(128, -1) for (o, n) in chunks]
        return np.concatenate(blocks, axis=1)

    def pkw(a):
        """[H, H] weightT -> [128, P, KT*128] (p-major slices)."""
        return np.ascontiguousarray(
            a.reshape(KT, 128, P, 128).transpose(1, 2, 0, 3)).reshape(128, P, -1)

    wqp = pkw((2.0 * np.asarray(Wq, dtype=np.float32).T).astype(bf))
    wkp = pkw((2.0 * np.asarray(Wk, dtype=np.float32).T).astype(bf))
    wvT = np.ascontiguousarray(
        (2.0 * np.asarray(Wv, dtype=np.float32).T).astype(bf)
        .reshape(KT, 128, 2, 384).transpose(1, 2, 0, 3)).reshape(128, -1)
    woT = np.ascontiguousarray(
        np.asarray(Wo, dtype=np.float32).T.astype(bf)
        .reshape(KT, 128, H).swapaxes(0, 1)).reshape(128, -1)

    in_maps = []
    for b in range(B):
        xb = 0.5 * x[b]                         # x' = 0.5 x  [L, H]
        idx = keep[b]
        n = len(idx)
        xkb = np.zeros((n_max, H), np.float32)  # compacted key rows of x'
        xkb[:n] = xb[idx]
        rvb = np.zeros((n_max, H), np.float32)  # rv = relu(x'_keys) + bv
        rvb[:n] = np.maximum(xb[idx], 0.0) + bv[None, :]
        # per-key mask bias: NEG for this batch's padding keys (n..n_max)
        # so exp() kills them exactly; real keys get 0
        valid = np.zeros((lk,), np.float32)
        valid[n:] = NEG
        mb_t = valid[:lkt * 128].reshape(lkt, 128).T

        xKa = pk_chunks(xkb.T.astype(bf), kchunks)      # [128, KT*n_max]
        xTa = pk_chunks(xb.T.astype(bf), [(0, 512), (512, 512)])

        wkm = np.concatenate(
            [wkp[:, 0], mb_t.astype(bf)], axis=1)
        rv_flat = np.ascontiguousarray(
            rvb[:(lkt - 1) * 128].astype(bf)
            .reshape(lkt - 1, 128, H).swapaxes(0, 1)).reshape(128, -1)
        rv_last = np.ascontiguousarray(rvb[(lkt - 1) * 128:n_max].astype(bf))
        # pad rv_last rows up to rows_last (program expects rows_last rows)
        if rv_last.shape[0] < rows_last:
            rv_last = np.concatenate(
                [rv_last, np.zeros((rows_last - rv_last.shape[0], H), bf)])

        in_maps.append({
            "wkm": np.ascontiguousarray(wkm),
            "wq0": np.ascontiguousarray(wqp[:, 0]),
            "xk": np.ascontiguousarray(xKa),
            "xt": np.ascontiguousarray(xTa),
            "wC": np.ascontiguousarray(wkp[:, 1:]),
            "wD": np.ascontiguousarray(wqp[:, 1:]),
            "wv": wvT, "wo": woT,
            "rv": rv_flat,
            "rvl": rv_last,
            **({"bo": bo} if has_bo else {}),
        })

    trace = bool(os.environ.get("BASS_KERNEL_TRACE"))
    res = run_bass_kernel_spmd(nc, in_maps, list(range(B)), trace=trace)
    LAST_EXEC_NS = res.exec_time_ns
    LAST_RESULTS = res
    return np.stack(
        [res.results[b]["out"].astype(np.float32) for b in range(B)], axis=0)


# revision 26
# speedup vs baseline: 1.1176x; 1.1176x over previous
"""BertSelfAttention (with value-bypass relu-add) on 8 Trainium2 NeuronCores.

Strategy: data-parallel over batch B=8 -> one batch element per core, no
collectives. Per core, attention is computed in a transposed-softmax layout:

  qT, kT = (x' @ W'.T).T + relu(x'.T)   [H, L]  (x' = 0.5x, W' = 2W, so
                                        x'@W'.T = x@W.T and relu(x') = 0.5relu(x);
                                        the residual add is ONE fused
                                        scalar_tensor_tensor relu+add -- no
                                        separate r tensors are ever sent)
  v      = x' @ Wv'.T + rv              [Lk, H], augmented with a ones column
  S.T    = kT_head.T-matmul             [lk, lq]  (keys on partitions)
  E      = exp(S.T * 1/8 + maskbias)    one N=2048 activation per (p, key-tile)
                                        covering both heads x both query halves
                                        (single merged 4-bank PSUM tile)
  PV     = [v_head | ones(64)].T @ E    -> rows 0..63 unnormalized attn.T,
                                         rows 64..127 = softmax denominator
                                         replicated 64x by the matmul itself
  attnT  = PV[0:64] * recip(PV[64:128]) (partition-shifted DVE copy out of
                                         PSUM + approx-recip + mul; no gpsimd
                                         broadcast on the critical chain)
  out    = attnT.T-matmul with Wo.T

Masked keys are compacted away on the host (gather unmasked key rows; only
the real nmax key columns are ever shipped or touched).

DMA: inputs stream over THREE queues (sync/scalar HWDGE + gpsimd SWDGE),
each transfer need-ordered so the first scores tile's operands land ~6us
after issue. xk/xt are packed chunk-major (projection-chunk major) so the
first projection chunk only needs the head of the stream. Warmup matmuls
keep the HAM clock warm until wave-1 lands.

Emission: the PE tape is [st-tile, fillers, ...] pumped from a FIFO
(qk(p+1) chunks, v-proj groups, pv(p) one block late) so the in-order PE
never blocks on an exp-gated matmul while ready work exists. Scores matmul
pairs (K=64) row-tile into the two 64-row PE halves and run concurrently.
pv groups are emitted j-half-major so out-proj can chase the last pv.
"""

import os
import sys

for _p in ("/opt/trn_rl_repo", "/root/.axon_site/_ro/trn_rl_repo"):
    if os.path.isdir(_p) and _p not in sys.path:
        sys.path.insert(0, _p)

import ml_dtypes
import numpy as np

import concourse.bacc as bacc
import concourse.bass as bass
import concourse.mybir as mybir
import concourse.tile as tile
from concourse.bass_utils import run_bass_kernel_spmd

B, L, H = 8, 1024, 768
NH, DH = 12, 64
P = H // 128             # 6 head-pair tiles
SCALE = 1.0 / 8.0
NEG = -1e9
KT = H // 128            # 6 contraction tiles over hidden dim
LQT = L // 128           # 8 query row-tiles
F32 = mybir.dt.float32
BF16 = mybir.dt.bfloat16
ALU = mybir.AluOpType

WARMUP_MM = 58           # N=128 dummy matmuls (~164ns each) until wave-1 lands
WARM_K1 = 8              # warm burst bridging the xk-c1 DMA stall
WARM_Q0 = 6              # warm burst bridging the xt-c0 DMA stall
SLOT_FILL = 1.7          # us of fillers pumped per (p, i) slot
SLOT_FILL0 = 2.2         # p == 0 slots

LAST_EXEC_NS = None
LAST_RESULTS = None
_CACHE = {}


def _chunks(total, maxc):
    """Split `total` into chunks of at most `maxc`; all boundaries are
    multiples of 128 so 128-wide key tiles never straddle a chunk."""
    n = -(-total // maxc)
    base = total // n
    base -= base % 128
    if base == 0:
        base = total
    sizes = [base] * n
    sizes[-1] = total - base * (n - 1)
    out, off = [], 0
    for s in sizes:
        out.append((off, s))
        off += s
    return out


def _build(lk, nmax, has_bo):
    """Build + compile the 8-core SPMD program; lk = padded key count
    (tile allocation), nmax = max real key count (compute bound)."""
    lkt = lk // 128          # key row-tiles
    rows_of = [min(128, nmax - 128 * i) for i in range(lkt)]
    rows_last = rows_of[-1]
    nc = bacc.Bacc("TRN2", target_bir_lowering=False, debug=False, num_devices=B)

    kchunks = _chunks(nmax, 512)     # k-side projection chunks
    nkc = len(kchunks)
    qchunks = [(0, 512), (512, 512)]

    # ---- DRAM inputs (all bf16) --------------------------------------
    wkm_d = nc.dram_tensor("wkm", [128, KT * 128 + lkt], BF16, kind="ExternalInput")
    wq0_d = nc.dram_tensor("wq0", [128, KT * 128], BF16, kind="ExternalInput")
    xk_d = nc.dram_tensor("xk", [128, KT * nmax], BF16, kind="ExternalInput")
    xt_d = nc.dram_tensor("xt", [128, 2 * KT * 512], BF16, kind="ExternalInput")
    wC_d = nc.dram_tensor("wC", [128, P - 1, KT * 128], BF16, kind="ExternalInput")
    wD_d = nc.dram_tensor("wD", [128, P - 1, KT * 128], BF16, kind="ExternalInput")
    wv_d = nc.dram_tensor("wv", [128, 2 * KT * 384], BF16, kind="ExternalInput")
    wo_d = nc.dram_tensor("wo", [128, KT * H], BF16, kind="ExternalInput")
    rv_d = nc.dram_tensor("rv", [128, (lkt - 1) * H], BF16, kind="ExternalInput")
    rvl_d = nc.dram_tensor("rvl", [rows_last, H], BF16, kind="ExternalInput")
    out_d = nc.dram_tensor("out", [L, H], BF16, kind="ExternalOutput")
    bo_d = nc.dram_tensor("bo", [H], F32, kind="ExternalInput") if has_bo else None

    exp_t = mybir.ActivationFunctionType.Exp

    with tile.TileContext(nc) as tc:
        with (
            tc.tile_pool(name="persist", bufs=1) as persist,
            tc.tile_pool(name="ep", bufs=2) as ep,
            tc.tile_pool(name="rcp", bufs=3) as rcp,
            tc.tile_pool(name="outp", bufs=1) as outp,
            tc.tile_pool(name="psum", bufs=1, space="PSUM") as psum,
        ):
            wkm = persist.tile([128, KT * 128 + lkt], BF16, tag="wkm", name="wkm")
            wq0 = persist.tile([128, KT * 128], BF16, tag="wq0", name="wq0")
            xk = persist.tile([128, KT * nmax], BF16, tag="xk", name="xk")
            xt = persist.tile([128, 2 * KT * 512], BF16, tag="xt", name="xt")
            wC = persist.tile([128, P - 1, KT * 128], BF16, tag="wC", name="wC")
            wD = persist.tile([128, P - 1, KT * 128], BF16, tag="wD", name="wD")
            wva = persist.tile([128, 2 * KT * 384], BF16, tag="wva", name="wva")
            woa = persist.tile([128, KT * H], BF16, tag="woa", name="woa")
            rva = persist.tile([128, lkt * H], BF16, tag="rva", name="rva")
            mbt = wkm[:, KT * 128:]

            # chunk-major views into xk / xt:
            # xk layout: [chunk c][tile k][cols of chunk c]
            xk_base = [KT * o for (o, n) in kchunks]

            def xkv(k, ci):
                o, n = kchunks[ci]
                b = xk_base[ci] + k * n
                return xk[:, b:b + n]

            def xkv_tile(k, lt, rows):
                """key-tile slice [lt*128, lt*128+rows) of x' tile k."""
                for ci, (o, n) in enumerate(kchunks):
                    if o <= lt * 128 < o + n:
                        b = xk_base[ci] + k * n + (lt * 128 - o)
                        return xk[:, b:b + rows]
                raise AssertionError

            def xtv(k, j):
                b = (j * KT + k) * 512
                return xt[:, b:b + 512]

            def wk_v(p, k):
                if p == 0:
                    return wkm[:, k * 128:(k + 1) * 128]
                return wC[:, p - 1, k * 128:(k + 1) * 128]

            def wq_v(p, k):
                if p == 0:
                    return wq0[:, k * 128:(k + 1) * 128]
                return wD[:, p - 1, k * 128:(k + 1) * 128]

            def wvt(ch, k):
                b = (ch * KT + k) * 384
                return wva[:, b:b + 384]

            woTt = [woa[:, k * H:(k + 1) * H] for k in range(KT)]

            # ---- input DMAs: 3 need-ordered queues -------------------
            # per-queue FIFO order is the transfer sequencer; each queue
            # sustains ~95 GB/s, so the critical wave is striped across
            # all three and late-needed bulk sits behind it.
            c0n = kchunks[0][1]
            nc.sync.dma_start(wkm[:], wkm_d[:])                       # wk0+mask
            nc.scalar.dma_start(wq0[:], wq0_d[:])
            nc.gpsimd.dma_start(xt[:, KT * 512:], xt_d[:, KT * 512:])  # xt c1
            nc.sync.dma_start(xk[:, 0:KT * c0n], xk_d[:, 0:KT * c0n])  # xk c0
            nc.scalar.dma_start(xt[:, 0:KT * 512], xt_d[:, 0:KT * 512])  # xt c0
            nc.gpsimd.dma_start(wva[:, 0:3 * 384], wv_d[:, 0:3 * 384])  # wv c0 k0-2
            if nkc > 1:
                nc.sync.dma_start(xk[:, KT * c0n:], xk_d[:, KT * c0n:])  # xk c1
            nc.sync.dma_start(wC[:, 0:1, :], wC_d[:, 0:1, :])
            nc.scalar.dma_start(wD[:, 0:1, :], wD_d[:, 0:1, :])
            nc.gpsimd.dma_start(wva[:, 6 * 384:9 * 384], wv_d[:, 6 * 384:9 * 384])
            if lkt > 1:
                nc.scalar.dma_start(rva[:, 0:H], rv_d[:, 0:H])        # rv t0
            nc.sync.dma_start(wva[:, 3 * 384:6 * 384], wv_d[:, 3 * 384:6 * 384])
            nc.gpsimd.dma_start(wva[:, 9 * 384:], wv_d[:, 9 * 384:])
            if lkt > 1:
                nc.sync.dma_start(rva[:, H:2 * H], rv_d[:, H:2 * H])  # rv t1
            if lkt > 2:
                nc.scalar.dma_start(rva[:, 2 * H:3 * H], rv_d[:, 2 * H:3 * H])
            if lkt > 3:
                nc.gpsimd.dma_start(rva[:, 3 * H:4 * H], rv_d[:, 3 * H:4 * H])
            nc.gpsimd.dma_start(
                rva[0:rows_last, (lkt - 1) * H:lkt * H], rvl_d[:])    # rv last
            nc.sync.dma_start(wC[:, 1:, :], wC_d[:, 1:, :])
            nc.scalar.dma_start(wD[:, 1:, :], wD_d[:, 1:, :])
            nc.gpsimd.dma_start(woa[:], wo_d[:])
            bo_bc = (persist.tile([128, H], F32, tag="bo", name="bo_bc")
                     if has_bo else None)
            if has_bo:
                bo_ap = bo_d.ap()
                nc.gpsimd.dma_start(
                    out=bo_bc[:],
                    in_=bass.AP(tensor=bo_ap.tensor, offset=0,
                                ap=[[0, 128], [1, H]]),
                )

            qTt = [persist.tile([128, L], BF16, tag=f"qT{i}", name=f"qT{i}")
                   for i in range(P)]
            kTt = [persist.tile([128, lk], BF16, tag=f"kT{i}", name=f"kT{i}")
                   for i in range(P)]
            vaug = [persist.tile([128, NH, 2 * DH], BF16, tag=f"va{i}", name=f"va{i}")
                    for i in range(lkt)]
            attnT = [persist.tile([128, L], BF16, tag=f"aT{i}", name=f"aT{i}")
                     for i in range(P)]
            # ones in cols DH..2DH of every head: the PV matmul then emits
            # the softmax denominator replicated across psum rows 64..127
            for lt in range(lkt):
                nc.gpsimd.memset(vaug[lt][0:rows_of[lt], :, DH:2 * DH], 1.0)

            # ---- PE warmup: dummy matmuls vs the HAM cold clock ------
            # N=128 keeps the granularity fine (~110ns/MM warm) so the last
            # warmup ends right as wave-1 lands; bursts bridge known DMA
            # stalls inside the prologue so the clock never re-throttles.
            warm = persist.tile([128, 512], BF16, tag="warm", name="warm")
            nc.vector.memset(warm[:], 0.125)

            def warmup(n):
                for _ in range(n):
                    pw = psum.tile([128, 512], F32, tag="ps", bufs=2,
                                   name="pwarm")
                    nc.tensor.matmul(pw[:, 0:128], warm[:, 0:128],
                                     warm[:, 0:128], start=True, stop=True)

            warmup(WARMUP_MM)

            # ---- v projection, natural layout [lk, H], augmented -----
            def emit_v_ch(lt, ch):
                rows = rows_of[lt]
                ps = psum.tile([128, 512], F32, tag="ps", bufs=2, name="psv")
                for k in range(KT):
                    nc.tensor.matmul(
                        ps[0:rows, 0:384],
                        xkv_tile(k, lt, rows),
                        wvt(ch, k),
                        start=(k == 0), stop=(k == KT - 1),
                    )
                nc.vector.tensor_add(
                    vaug[lt][0:rows, ch * 6:(ch + 1) * 6, 0:DH],
                    ps[0:rows, 0:384].rearrange("p (h d) -> p h d", d=DH),
                    rva[0:rows, lt * H + ch * 384:lt * H + (ch + 1) * 384]
                    .rearrange("p (h d) -> p h d", d=DH),
                )

            def emit_qk_grp(p, side, ci):
                """One projection chunk for head-pair p; side 0=k, 1=q.
                dst = relu(x'_p chunk) + sum_k w'(p,k).T @ x'_k chunk."""
                if side == 0:
                    o0, on = kchunks[ci]
                    wf, dst = wk_v, kTt
                    rhs = [xkv(k, ci) for k in range(KT)]
                    res = xkv(p, ci)
                else:
                    o0, on = qchunks[ci]
                    wf, dst = wq_v, qTt
                    rhs = [xtv(k, ci) for k in range(KT)]
                    res = xtv(p, ci)
                ps = psum.tile([128, 512], F32, tag="ps", bufs=2, name="psq")
                for k in range(KT):
                    nc.tensor.matmul(
                        ps[:, 0:on],
                        wf(p, k),
                        rhs[k],
                        start=(k == 0), stop=(k == KT - 1),
                    )
                # dst = (x' max 0) + ps   (r = relu(x') fused, no r tensor)
                nc.vector.scalar_tensor_tensor(
                    out=dst[p][:, o0:o0 + on],
                    in0=res,
                    scalar=0.0,
                    in1=ps[:, 0:on],
                    op0=ALU.max,
                    op1=ALU.add,
                )

            def qk_grps(p):
                out = [(0.0025 * on, lambda ci=ci: emit_qk_grp(p, 0, ci))
                       for ci, (o0, on) in enumerate(kchunks)]
                out += [(1.28, lambda ci=ci: emit_qk_grp(p, 1, ci))
                        for ci in (0, 1)]
                return out

            def emit_qk(p):
                for _, fn in qk_grps(p):
                    fn()

            def emit_st_i(p, i, ex):
                """Scores + exp for head pair p, key-tile i.
                TWO j-half PSUM tiles [128,1024] (tag bufs=2 -> 4 banks):
                act_j fires after just that half's two score MMs, and the
                next slot's j0 MMs only wait on act_j0 of this slot, so
                the scalar exp stream runs back-to-back while scores/
                fillers keep the PE dense."""
                rows = rows_of[i]
                ext = ep.tile([128, 2048], BF16, tag=f"ex_{i}", name=f"ex_{i}")
                ex[i] = ext

                for j in range(2):
                    pm = psum.tile([128, 1024], F32, tag="st", bufs=2,
                                   name="st_ps")
                    for hh, off in ((0, 0), (1, 64)):
                        nc.tensor.matmul(
                            pm[0:rows, hh * 512:(hh + 1) * 512],
                            kTt[p][off:off + DH, i * 128:i * 128 + rows],
                            qTt[p][off:off + DH, j * 512:(j + 1) * 512],
                            start=True, stop=True,
                        )
                    nc.scalar.activation(
                        ext[0:rows, j * 1024:(j + 1) * 1024],
                        pm[0:rows, :], exp_t,
                        bias=mbt[0:rows, i:i + 1], scale=SCALE)

            def emit_pv_grp(p, ex, hh, j):
                """PV + normalization for head 2p+hh, query half j."""
                off = 64 * hh
                head = 2 * p + hh
                pv = psum.tile([128, 512], F32, tag="pv", bufs=2,
                               name="pv_ps")
                # 18-row remainder tile first: its extra overhead merges
                # into the group-leading pipeline restart
                order = [lkt - 1] + list(range(lkt - 1))
                for oi, i in enumerate(order):
                    rows = rows_of[i]
                    nc.tensor.matmul(
                        pv[:],
                        vaug[i][0:rows, head, :],
                        ex[i][0:rows, j * 1024 + hh * 512:j * 1024 + (hh + 1) * 512],
                        start=(oi == 0), stop=(oi == lkt - 1),
                    )
                # custom-DVE recip can't read PSUM (HW): shifted copy out
                dn = rcp.tile([DH, 512], F32, tag="dn", name="dn_t")
                nc.vector.tensor_copy(dn[:], pv[DH:2 * DH, :])
                rc = rcp.tile([DH, 512], F32, tag="rc", name="rc_t")
                nc.vector.reciprocal_approx_fast(out=rc[:], in_=dn[:])
                nc.vector.tensor_mul(
                    attnT[p][off:off + DH, j * 512:(j + 1) * 512],
                    pv[0:DH, :], rc[:])

            # ---- out-proj split: k=0..4 partials (outpre) are PE filler
            # for the last head-pair's exp phase; only the k=5 matmul +
            # add + store remain after the final attnT lands.
            so_tiles = {}

            def emit_outpre(lt, gi):
                o0, on = ((0, 512), (512, 256))[gi]
                if lt not in so_tiles:
                    so_tiles[lt] = outp.tile([128, H], BF16, tag=f"so{lt}",
                                             bufs=1, name=f"so{lt}")
                so = so_tiles[lt]
                ps = psum.tile([128, 512], F32, tag="ps", bufs=2, name="pre")
                for k in range(KT - 1):
                    nc.tensor.matmul(
                        ps[:, 0:on],
                        attnT[k][:, lt * 128:(lt + 1) * 128],
                        woTt[k][:, o0:o0 + on],
                        start=(k == 0), stop=(k == KT - 2),
                    )
                if has_bo:
                    nc.vector.tensor_add(
                        so[:, o0:o0 + on], ps[:, 0:on], bo_bc[:, o0:o0 + on])
                else:
                    nc.vector.tensor_copy(so[:, o0:o0 + on], ps[:, 0:on])

            def emit_outfin(lt):
                so = so_tiles[lt]
                for gi, (o0, on) in enumerate(((0, 512), (512, 256))):
                    if (2 * lt + gi) % 2 == 0:
                        ps = psum.tile([128, 512], F32, tag="ps", bufs=2,
                                       name="fc")
                    else:
                        ps = psum.tile([128, 1024], F32, tag="st", bufs=2,
                                       name="fc2")
                    nc.tensor.matmul(
                        ps[:, 0:on],
                        attnT[KT - 1][:, lt * 128:(lt + 1) * 128],
                        woTt[KT - 1][:, o0:o0 + on],
                        start=True, stop=True,
                    )
                    nc.vector.tensor_add(
                        so[:, o0:o0 + on], ps[:, 0:on], so[:, o0:o0 + on])
                if lt < LQT - 1:
                    eng = nc.sync if lt % 2 == 0 else nc.scalar
                    eng.dma_start(out_d[lt * 128:(lt + 1) * 128, :], so[:])
                else:
                    # split the last store across the rings: it is the tail
                    nc.sync.dma_start(
                        out_d[lt * 128:(lt + 1) * 128, 0:256], so[:, 0:256])
                    nc.scalar.dma_start(
                        out_d[lt * 128:(lt + 1) * 128, 256:512], so[:, 256:512])
                    nc.gpsimd.dma_start(
                        out_d[lt * 128:(lt + 1) * 128, 512:], so[:, 512:])

            # ---- prologue: qk(0), warm bursts bridging DMA stalls ----
            grps0 = [fn for _, fn in qk_grps(0)]
            grps0[0]()             # k-side chunk 0
            warmup(WARM_K1)
            for ci in range(1, nkc):
                grps0[ci]()        # k-side chunk 1
            warmup(WARM_Q0)
            grps0[nkc]()           # q-side chunk 0
            grps0[nkc + 1]()       # q-side chunk 1

            # The PE tape is [st-tile, fillers, st-tile, ...] so in-order
            # execution never blocks on a gated instruction while ready
            # work exists behind it. Fillers: qk(1) first (its weights
            # land first), then v-proj groups, qk(p+2) later, pv(p) one
            # block late (always dep-safe).
            exs = [dict() for _ in range(P)]
            fifo = []
            fifo.extend((c, ('qk', 1), fn) for c, fn in qk_grps(1))
            for lt in range(lkt):
                for ch in range(2):
                    fifo.append((1.0, None,
                                 lambda lt=lt, ch=ch: emit_v_ch(lt, ch)))

            def pump(budget):
                while fifo and budget > 0:
                    c, _, fn = fifo.pop(0)
                    fn()
                    budget -= c

            for p in range(P):
                # same-engine ordering: qk(p)'s PE matmuls must precede
                # st(p)'s in the PE stream -- drain them (and everything
                # queued before them) from the fifo now
                while any(t == ('qk', p) for _, t, _ in fifo):
                    _, _, fn = fifo.pop(0)
                    fn()
                for i in range(lkt):
                    emit_st_i(p, i, exs[p])
                    pump(SLOT_FILL0 if p == 0 else SLOT_FILL)
                if p + 2 < P:
                    fifo.extend((c, ('qk', p + 2), fn)
                                for c, fn in qk_grps(p + 2))
                # j-half-major so the last p's j=0 attnT completes first;
                # the final p's pv groups go to the FRONT of the queue so
                # leftover outpre work doesn't delay the critical chain
                grps = [(1.15, None, lambda p=p, hh=hh, j=j:
                         emit_pv_grp(p, exs[p], hh, j))
                        for j in (0, 1) for hh in (0, 1)]
                if p == P - 1:
                    fifo[0:0] = grps
                else:
                    fifo.extend(grps)
            pump(1e9)

            # ---------------- output projection ----------------
            for lt in range(LQT):
                so = outp.tile([128, H], BF16, tag="so", bufs=4, name="so_t")
                for gi, (o0, on) in enumerate(((0, 512), (512, 256))):
                    # alternate psum tags for a 4-deep psum rotation
                    if (2 * lt + gi) % 2 == 0:
                        ps = psum.tile([128, 512], F32, tag="ps", bufs=2,
                                       name="pc")
                    else:
                        ps = psum.tile([128, 1024], F32, tag="st", bufs=2,
                                       name="pc2")
                    for k in range(KT):
                        nc.tensor.matmul(
                            ps[:, 0:on],
                            attnT[k][:, lt * 128:(lt + 1) * 128],
                            woTt[k][:, o0:o0 + on],
                            start=(k == 0), stop=(k == KT - 1),
                        )
                    if has_bo:
                        nc.vector.tensor_add(
                            so[:, o0:o0 + on], ps[:, 0:on], bo_bc[:, o0:o0 + on])
                    elif (2 * lt + gi) % 2 == 0:
                        nc.scalar.copy(so[:, o0:o0 + on], ps[:, 0:on])
                    else:
                        nc.vector.tensor_copy(so[:, o0:o0 + on], ps[:, 0:on])
                if lt < LQT - 1:
                    eng = nc.sync if lt % 2 == 0 else nc.scalar
                    eng.dma_start(out_d[lt * 128:(lt + 1) * 128, :], so[:])
                else:
                    # split the last store across the rings: it is the tail
                    nc.sync.dma_start(
                        out_d[lt * 128:(lt + 1) * 128, 0:256], so[:, 0:256])
                    nc.scalar.dma_start(
                        out_d[lt * 128:(lt + 1) * 128, 256:512], so[:, 256:512])
                    nc.gpsimd.dma_start(
                        out_d[lt * 128:(lt + 1) * 128, 512:], so[:, 512:])

    nc.compile()
    return nc


def kernel(hidden_states, attention_mask, Wq, bq, Wk, bk, Wv, bv, Wo, bo):
    global LAST_EXEC_NS, LAST_RESULTS
    x = np.ascontiguousarray(np.asarray(hidden_states, dtype=np.float32))
    mask = np.asarray(attention_mask).astype(bool).reshape(B, L)
    bq = np.asarray(bq, dtype=np.float32)
    bk = np.asarray(bk, dtype=np.float32)
    bv = np.asarray(bv, dtype=np.float32)
    bo = np.asarray(bo, dtype=np.float32)
    has_bo = bool(np.any(bo))
    assert not (np.any(bq) or np.any(bk)), \
        "fast path assumes zero bq/bk (relu fusion)"

    keep = [np.nonzero(~mask[b])[0] for b in range(B)]
    n_max = max(max(len(k) for k in keep), 128)
    lk = -(-n_max // 128) * 128   # padded key count, multiple of 128
    lkt = lk // 128
    rows_last = n_max - 128 * (lkt - 1)

    key = (lk, n_max, has_bo)
    if key not in _CACHE:
        _CACHE[key] = _build(lk, n_max, has_bo)
    nc = _CACHE[key]

    bf = ml_dtypes.bfloat16
    kchunks = _chunks(n_max, 512)

    def pk_chunks(a, chunks):
        """[H, X] -> chunk-major [128, sum(KT*n_c)]: for each chunk, the
        six 128-row tiles' chunk columns, tile-major."""
        t = a.reshape(KT, 128, a.shape[1])
        blocks = [np.ascontiguousarray(t[:, :, o:o + n].swapaxes(0, 1))
                  .reshape(128, -1) for (o, n) in chunks]
        return np.concatenate(blocks, axis=1)

    def pkw(a):
        """[H, H] weightT -> [128, P, KT*128] (p-major slices)."""
        return np.ascontiguousarray(
            a.reshape(KT, 128, P, 128).transpose(1, 2, 0, 3)).reshape(128, P, -1)

    wqp = pkw((2.0 * np.asarray(Wq, dtype=np.float32).T).astype(bf))
    wkp = pkw((2.0 * np.asarray(Wk, dtype=np.float32).T).astype(bf))
    wvT = np.ascontiguousarray(
        (2.0 * np.asarray(Wv, dtype=np.float32).T).astype(bf)
        .reshape(KT, 128, 2, 384).transpose(1, 2, 0, 3)).reshape(128, -1)
    woT = np.ascontiguousarray(
        np.asarray(Wo, dtype=np.float32).T.astype(bf)
        .reshape(KT, 128, H).swapaxes(0, 1)).reshape(128, -1)

    in_maps = []
    for b in range(B):
        xb = 0.5 * x[b]                         # x' = 0.5 x  [L, H]
        idx = keep[b]
        n = len(idx)
        xkb = np.zeros((n_max, H), np.float32)  # compacted key rows of x'
        xkb[:n] = xb[idx]
        rvb = np.zeros((n_max, H), np.float32)  # rv = relu(x'_keys) + bv
        rvb[:n] = np.maximum(xb[idx], 0.0) + bv[None, :]
        # per-key mask bias: NEG for this batch's padding keys (n..n_max)
        # so exp() kills them exactly; real keys get 0
        valid = np.zeros((lk,), np.float32)
        valid[n:] = NEG
        mb_t = valid[:lkt * 128].reshape(lkt, 128).T

        xKa = pk_chunks(xkb.T.astype(bf), kchunks)      # [128, KT*n_max]
        xTa = pk_chunks(xb.T.astype(bf), [(0, 512), (512, 512)])

        wkm = np.concatenate(
            [wkp[:, 0], mb_t.astype(bf)], axis=1)
        rv_flat = np.ascontiguousarray(
            rvb[:(lkt - 1) * 128].astype(bf)
            .reshape(lkt - 1, 128, H).swapaxes(0, 1)).reshape(128, -1)
        rv_last = np.ascontiguousarray(rvb[(lkt - 1) * 128:n_max].astype(bf))
        # pad rv_last rows up to rows_last (program expects rows_last rows)
        if rv_last.shape[0] < rows_last:
            rv_last = np.concatenate(
                [rv_last, np.zeros((rows_last - rv_last.shape[0], H), bf)])

        in_maps.append({
            "wkm": np.ascontiguousarray(wkm),
            "wq0": np.ascontiguousarray(wqp[:, 0]),
            "xk": np.ascontiguousarray(xKa),
            "xt": np.ascontiguousarray(xTa),
            "wC": np.ascontiguousarray(wkp[:, 1:]),
            "wD": np.ascontiguousarray(wqp[:, 1:]),
            "wv": wvT, "wo": woT,
            "rv": rv_flat,
            "rvl": rv_last,
            **({"bo": bo} if has_bo else {}),
        })

    trace = bool(os.environ.get("BASS_KERNEL_TRACE"))
    res = run_bass_kernel_spmd(nc, in_maps, list(range(B)), trace=trace)
    LAST_EXEC_NS = res.exec_time_ns
    LAST_RESULTS = res
    return np.stack(
        [res.results[b]["out"].astype(np.float32) for b in range(B)], axis=0)
